# revision 6
# baseline (speedup 1.0000x reference)
"""GAT layer (4 heads, 128 dim) on 8 Trainium2 NeuronCores.

Strategy (edge-parallel over dst, degree-sorted):
  - Host relabels nodes by descending in-degree and deals them round-robin
    to the 8 cores, so every core sees an identical degree profile and the
    SPMD program (one Bass module, per-core data) can bake a shared
    per-window max-degree schedule with ~2% slot padding.
  - Each core builds the full K|V table (bf16, [N_pad, 256]) from x with
    TensorE matmuls, then for each 128-node window gathers its edges'
    K|V rows (512B each, indirect DMA), computes per-edge scores
    (q.k/sqrt(d)), exp, masks padding slots, and reduces messages and
    score-sums per node with strided VectorE reductions.  The global-max
    shift of the reference cancels in the normalization (up to the 1e-8
    eps, relative error ~1e-8), so it is skipped.
  - attention normalization is applied after aggregation:
    out_n = (sum_e exp_e * V_src) / (sum_e exp_e + 1e-8)  == reference.
  - Final per-window: transpose, out-projection matmul, bias, relu.
  - No collectives: each core owns a disjoint slice of output rows; the
    host scatters per-core outputs back through the permutation.
"""

import os
import sys

for _p in ("/opt/trn_rl_repo", "/opt/pypackages"):
    if _p not in sys.path:
        sys.path.append(_p)

import numpy as np
import ml_dtypes

P = 128
N_CORES = 8
DIM = 128
NUM_HEADS = 4
HEAD_DIM = 32
INV_SQRT_HD = 1.0 / np.sqrt(HEAD_DIM).astype(np.float32)
EPS = 1e-8
D_CH = 16          # slots per gather chunk
PH1_CHUNK = 512    # nodes per phase-1 x-chunk

_PROGRAM_CACHE = {}


def _make_chunks(d):
    """Split d slots into chunks of <= D_CH."""
    out = []
    s = 0
    while s < d:
        out.append((s, min(D_CH, d - s)))
        s += D_CH
    return out


def _build_program(n_pad, n_c, d_sched, c_tot):
    import concourse.bass as bass
    import concourse.bacc as bacc
    import concourse.mybir as mybir
    from concourse.tile import TileContext
    from concourse.masks import make_identity

    f32 = mybir.dt.float32
    bf16 = mybir.dt.bfloat16
    i32 = mybir.dt.int32
    n_w = len(d_sched)
    d_max = max(max(d_sched), D_CH)
    offs = np.concatenate([[0], np.cumsum(d_sched)]).astype(int)

    nc = bacc.Bacc()
    xT_full = nc.dram_tensor("xT_full", [P, n_pad], bf16, kind="ExternalInput")
    xT_q = nc.dram_tensor("xT_q", [P, n_c], bf16, kind="ExternalInput")
    w_qT = nc.dram_tensor("w_qT", [P, DIM], bf16, kind="ExternalInput")
    w_kT = nc.dram_tensor("w_kT", [P, DIM], bf16, kind="ExternalInput")
    w_vT = nc.dram_tensor("w_vT", [P, DIM], bf16, kind="ExternalInput")
    w_oT = nc.dram_tensor("w_oT", [P, DIM], bf16, kind="ExternalInput")
    b_q = nc.dram_tensor("b_q", [P, DIM], f32, kind="ExternalInput")
    b_k = nc.dram_tensor("b_k", [P, DIM], f32, kind="ExternalInput")
    b_v = nc.dram_tensor("b_v", [P, DIM], f32, kind="ExternalInput")
    b_o = nc.dram_tensor("b_o", [P, DIM], f32, kind="ExternalInput")
    src_tab = nc.dram_tensor("src_tab", [P, max(c_tot, 1)], i32, kind="ExternalInput")
    deg_tab = nc.dram_tensor("deg_tab", [P, n_w], f32, kind="ExternalInput")
    out = nc.dram_tensor("out", [n_c, DIM], f32, kind="ExternalOutput")
    kv_dram = nc.dram_tensor("kv_dram", [n_pad, 2 * DIM], bf16)

    with TileContext(nc) as tc:
        with (
            tc.tile_pool(name="consts", bufs=1) as cp,
            tc.tile_pool(name="ph1", bufs=3) as p1,
            tc.tile_pool(name="ph1ps", bufs=2, space="PSUM") as p1ps,
            tc.tile_pool(name="win", bufs=3) as wp,
            tc.tile_pool(name="winacc", bufs=2) as ap,
            tc.tile_pool(name="winps", bufs=1, space="PSUM") as pp,
        ):
            # ---- constants ----
            wq_sb = cp.tile([P, DIM], bf16, tag="wq")
            wk_sb = cp.tile([P, DIM], bf16, tag="wk")
            wv_sb = cp.tile([P, DIM], bf16, tag="wv")
            wo_sb = cp.tile([P, DIM], bf16, tag="wo")
            bq_sb = cp.tile([P, DIM], f32, tag="bq")
            bk_sb = cp.tile([P, DIM], f32, tag="bk")
            bv_sb = cp.tile([P, DIM], f32, tag="bv")
            bo_sb = cp.tile([P, DIM], f32, tag="bo")
            nc.sync.dma_start(out=wq_sb[:], in_=w_qT[:])
            nc.sync.dma_start(out=wk_sb[:], in_=w_kT[:])
            nc.sync.dma_start(out=wv_sb[:], in_=w_vT[:])
            nc.sync.dma_start(out=wo_sb[:], in_=w_oT[:])
            nc.sync.dma_start(out=bq_sb[:], in_=b_q[:])
            nc.sync.dma_start(out=bk_sb[:], in_=b_k[:])
            nc.sync.dma_start(out=bv_sb[:], in_=b_v[:])
            nc.sync.dma_start(out=bo_sb[:], in_=b_o[:])
            src_sb = cp.tile([P, max(c_tot, 1)], i32, tag="src")
            deg_sb = cp.tile([P, n_w], f32, tag="deg")
            nc.sync.dma_start(out=src_sb[:], in_=src_tab[:])
            nc.sync.dma_start(out=deg_sb[:], in_=deg_tab[:])
            xq_sb = cp.tile([P, n_c], bf16, tag="xq")
            nc.sync.dma_start(out=xq_sb[:], in_=xT_q[:])
            ident = cp.tile([P, P], f32, tag="ident")
            make_identity(nc, ident[:])
            iota_i = cp.tile([P, d_max], i32, tag="iotai")
            nc.gpsimd.iota(iota_i[:], pattern=[[1, d_max]], base=0,
                           channel_multiplier=0)
            iota_f = cp.tile([P, d_max], f32, tag="iotaf")
            nc.vector.tensor_copy(out=iota_f[:], in_=iota_i[:])
            relu_bo = cp.tile([P, DIM], f32, tag="relubo")
            nc.scalar.activation(out=relu_bo[:], in_=bo_sb[:],
                                 func=mybir.ActivationFunctionType.Relu)

            # ---- phase 1: K|V table ----
            for c0 in range(0, n_pad, PH1_CHUNK):
                cw = min(PH1_CHUNK, n_pad - c0)
                xc = p1.tile([P, cw], bf16, tag="xc")
                nc.sync.dma_start(out=xc[:], in_=xT_full[:, c0:c0 + cw])
                for s0 in range(0, cw, P):
                    kv_sb = p1.tile([P, 2 * DIM], bf16, tag="kvsb")
                    ps_k = p1ps.tile([P, DIM], f32, tag="psk")
                    ps_v = p1ps.tile([P, DIM], f32, tag="psv")
                    lhs = xc[:, s0:s0 + P]
                    nc.tensor.matmul(out=ps_k[:], lhsT=lhs, rhs=wk_sb[:],
                                     start=True, stop=True)
                    nc.tensor.matmul(out=ps_v[:], lhsT=lhs, rhs=wv_sb[:],
                                     start=True, stop=True)
                    nc.vector.tensor_tensor(out=kv_sb[:, 0:DIM], in0=ps_k[:],
                                            in1=bk_sb[:], op=mybir.AluOpType.add)
                    nc.vector.tensor_tensor(out=kv_sb[:, DIM:2 * DIM], in0=ps_v[:],
                                            in1=bv_sb[:], op=mybir.AluOpType.add)
                    node0 = c0 + s0
                    nc.sync.dma_start(out=kv_dram[node0:node0 + P, :], in_=kv_sb[:])

            tc.strict_bb_all_engine_barrier()

            # ---- phase 2: windows ----
            for w in range(n_w):
                d_w = d_sched[w]
                row0 = w * P
                if d_w == 0:
                    nc.sync.dma_start(out=out[row0:row0 + P, :], in_=relu_bo[:])
                    continue

                # q_w = xq[:, window] @ WqT + bq   (node-major, bf16)
                ps_q = pp.tile([P, DIM], f32, tag="psq")
                nc.tensor.matmul(out=ps_q[:], lhsT=xq_sb[:, row0:row0 + P],
                                 rhs=wq_sb[:], start=True, stop=True)
                q_w = wp.tile([P, DIM], bf16, tag="qw")
                nc.vector.tensor_tensor(out=q_w[:], in0=ps_q[:], in1=bq_sb[:],
                                        op=mybir.AluOpType.add)

                agg = ap.tile([P, DIM], f32, tag="agg")
                ssum = ap.tile([P, NUM_HEADS], f32, tag="ssum")
                nc.vector.memset(agg[:], 0.0)
                nc.vector.memset(ssum[:], 0.0)

                for (s0, sl) in _make_chunks(d_w):
                    col0 = int(offs[w]) + s0
                    kv_g = wp.tile([P, D_CH * 2 * DIM], bf16, tag="kvg")
                    # HW indirect DMA honors one offset per partition: one
                    # gather instruction per edge slot.
                    for s in range(sl):
                        nc.gpsimd.indirect_dma_start(
                            out=kv_g[:, s * 2 * DIM:(s + 1) * 2 * DIM],
                            out_offset=None,
                            in_=kv_dram[:],
                            in_offset=bass.IndirectOffsetOnAxis(
                                ap=src_sb[:, col0 + s:col0 + s + 1], axis=0),
                        )
                    kv3 = kv_g[:, :sl * 2 * DIM].rearrange(
                        "p (s c) -> p s c", c=2 * DIM)

                    # scores: t = K * q  -> reduce over head_dim
                    tmul = wp.tile([P, D_CH * DIM], bf16, tag="tmul")
                    t3 = tmul[:, :sl * DIM].rearrange("p (s f) -> p s f", f=DIM)
                    nc.vector.tensor_tensor(
                        out=t3, in0=kv3[:, :, 0:DIM],
                        in1=q_w[:, None, :].broadcast_to([P, sl, DIM]),
                        op=mybir.AluOpType.mult)
                    scr = wp.tile([P, D_CH * NUM_HEADS], f32, tag="scr")
                    nc.vector.tensor_reduce(
                        out=scr[:, :sl * NUM_HEADS],
                        in_=tmul[:, :sl * DIM].rearrange(
                            "p (s h d) -> p s h d", h=NUM_HEADS, d=HEAD_DIM),
                        op=mybir.AluOpType.add, axis=mybir.AxisListType.X)

                    # exp(score/sqrt(hd)) * validity mask
                    exps = wp.tile([P, D_CH * NUM_HEADS], f32, tag="exps")
                    nc.scalar.activation(
                        out=exps[:, :sl * NUM_HEADS], in_=scr[:, :sl * NUM_HEADS],
                        func=mybir.ActivationFunctionType.Exp, scale=float(INV_SQRT_HD))
                    mask = wp.tile([P, D_CH], f32, tag="mask")
                    nc.gpsimd.tensor_scalar(
                        out=mask[:, :sl], in0=iota_f[:, s0:s0 + sl],
                        scalar1=deg_sb[:, w:w + 1], scalar2=None,
                        op0=mybir.AluOpType.is_lt)
                    expm = wp.tile([P, D_CH * NUM_HEADS], f32, tag="expm")
                    nc.vector.tensor_tensor(
                        out=expm[:, :sl * NUM_HEADS].rearrange(
                            "p (s h) -> p s h", h=NUM_HEADS),
                        in0=exps[:, :sl * NUM_HEADS].rearrange(
                            "p (s h) -> p s h", h=NUM_HEADS),
                        in1=mask[:, :sl, None].broadcast_to([P, sl, NUM_HEADS]),
                        op=mybir.AluOpType.mult)

                    # score sums (reduce over slots)
                    ssc = wp.tile([P, NUM_HEADS], f32, tag="ssc")
                    nc.vector.tensor_reduce(
                        out=ssc[:],
                        in_=expm[:, :sl * NUM_HEADS].rearrange(
                            "p (s h) -> p s h", h=NUM_HEADS).transpose([0, 2, 1]),
                        op=mybir.AluOpType.add, axis=mybir.AxisListType.X)
                    nc.vector.tensor_tensor(out=ssum[:], in0=ssum[:], in1=ssc[:],
                                            op=mybir.AluOpType.add)

                    # messages: m = V * expm ; aggregate over slots
                    msm = wp.tile([P, D_CH * DIM], bf16, tag="msm")
                    m4 = msm[:, :sl * DIM].rearrange(
                        "p (s h d) -> p s h d", h=NUM_HEADS, d=HEAD_DIM)
                    nc.vector.tensor_tensor(
                        out=m4,
                        in0=kv3[:, :, DIM:2 * DIM].rearrange(
                            "p s (h d) -> p s h d", d=HEAD_DIM),
                        in1=expm[:, :sl * NUM_HEADS].rearrange(
                            "p (s h) -> p s h", h=NUM_HEADS)[:, :, :, None]
                            .broadcast_to([P, sl, NUM_HEADS, HEAD_DIM]),
                        op=mybir.AluOpType.mult)
                    agc = wp.tile([P, DIM], f32, tag="agc")
                    nc.vector.tensor_reduce(
                        out=agc[:],
                        in_=msm[:, :sl * DIM].rearrange(
                            "p (s f) -> p s f", f=DIM).transpose([0, 2, 1]),
                        op=mybir.AluOpType.add, axis=mybir.AxisListType.X)
                    nc.vector.tensor_tensor(out=agg[:], in0=agg[:], in1=agc[:],
                                            op=mybir.AluOpType.add)

                # normalize: agg / (ssum + eps), per head
                inv4 = wp.tile([P, NUM_HEADS], f32, tag="inv4")
                nc.vector.tensor_scalar(
                    out=inv4[:], in0=ssum[:], scalar1=float(EPS), scalar2=None,
                    op0=mybir.AluOpType.add)
                nc.vector.reciprocal(out=inv4[:], in_=inv4[:])
                aggn = wp.tile([P, DIM], f32, tag="aggn")
                nc.vector.tensor_tensor(
                    out=aggn[:].rearrange("p (h d) -> p h d", d=HEAD_DIM),
                    in0=agg[:].rearrange("p (h d) -> p h d", d=HEAD_DIM),
                    in1=inv4[:, :, None].broadcast_to([P, NUM_HEADS, HEAD_DIM]),
                    op=mybir.AluOpType.mult)

                # out = relu(aggn @ WoT + bo)
                ps_t = pp.tile([P, DIM], f32, tag="pst")
                nc.tensor.transpose(out=ps_t[:], in_=aggn[:], identity=ident[:])
                aggT = wp.tile([P, DIM], bf16, tag="aggT")
                nc.scalar.copy(out=aggT[:], in_=ps_t[:])
                ps_o = pp.tile([P, DIM], f32, tag="pso")
                nc.tensor.matmul(out=ps_o[:], lhsT=aggT[:], rhs=wo_sb[:],
                                 start=True, stop=True)
                res = wp.tile([P, DIM], f32, tag="res")
                nc.vector.tensor_tensor(out=res[:], in0=ps_o[:], in1=bo_sb[:],
                                        op=mybir.AluOpType.add)
                res2 = wp.tile([P, DIM], f32, tag="res2")
                nc.scalar.activation(out=res2[:], in_=res[:],
                                     func=mybir.ActivationFunctionType.Relu)
                nc.sync.dma_start(out=out[row0:row0 + P, :], in_=res2[:])

    return nc


def prepare(x, edge_index, Wq, bq, Wk, bk, Wv, bv, Wo, bo):
    """Host-side layout prep: permutation, dealing, slot tables. No math."""
    n = x.shape[0]
    e = edge_index.shape[1]
    n_c = -(-n // (N_CORES * P)) * P
    n_pad = N_CORES * n_c
    n_w = n_c // P

    src = np.asarray(edge_index[0], dtype=np.int64)
    dst = np.asarray(edge_index[1], dtype=np.int64)
    deg = np.bincount(dst, minlength=n_pad).astype(np.int64)

    order = np.argsort(-deg, kind="stable")          # rank -> node
    deg_sorted = deg[order]
    rank_of = np.empty(n_pad, dtype=np.int64)
    rank_of[order] = np.arange(n_pad)

    node_at = order.reshape(n_c, N_CORES).T          # [core, pos] -> node
    d_sched = tuple(int(deg_sorted[w * P * N_CORES]) for w in range(n_w))
    offs = np.concatenate([[0], np.cumsum(d_sched)]).astype(np.int64)
    c_tot = int(offs[-1])

    # per-(core,partition,col) src table, vectorized over edges
    e_order = np.argsort(dst, kind="stable")
    dst_s = dst[e_order]
    src_s = src[e_order]
    starts = np.zeros(n_pad + 1, dtype=np.int64)
    np.cumsum(np.bincount(dst_s, minlength=n_pad), out=starts[1:])
    j = np.arange(e) - starts[dst_s]                 # intra-node slot
    r = rank_of[dst_s]
    m = r % N_CORES
    pos = r // N_CORES
    w_arr = pos // P
    p_arr = pos % P
    col = offs[w_arr] + j
    src_tabs = np.zeros((N_CORES, P, max(c_tot, 1)), dtype=np.int32)
    flat = src_tabs.reshape(N_CORES, -1)
    flat[m, p_arr * max(c_tot, 1) + col] = src_s.astype(np.int32)

    deg_tabs = deg[node_at].reshape(N_CORES, n_w, P).transpose(0, 2, 1) \
        .astype(np.float32)                          # [core, p, w]
    deg_tabs = np.ascontiguousarray(deg_tabs)

    xpad = np.zeros((n_pad, DIM), dtype=np.float32)
    xpad[:n] = np.asarray(x, dtype=np.float32)
    xT_full = np.ascontiguousarray(xpad.T).astype(ml_dtypes.bfloat16)

    in_maps = []
    common = {
        "xT_full": xT_full,
        "w_qT": np.ascontiguousarray(np.asarray(Wq, np.float32).T).astype(ml_dtypes.bfloat16),
        "w_kT": np.ascontiguousarray(np.asarray(Wk, np.float32).T).astype(ml_dtypes.bfloat16),
        "w_vT": np.ascontiguousarray(np.asarray(Wv, np.float32).T).astype(ml_dtypes.bfloat16),
        "w_oT": np.ascontiguousarray(np.asarray(Wo, np.float32).T).astype(ml_dtypes.bfloat16),
        "b_q": np.broadcast_to(np.asarray(bq, np.float32), (P, DIM)).copy(),
        "b_k": np.broadcast_to(np.asarray(bk, np.float32), (P, DIM)).copy(),
        "b_v": np.broadcast_to(np.asarray(bv, np.float32), (P, DIM)).copy(),
        "b_o": np.broadcast_to(np.asarray(bo, np.float32), (P, DIM)).copy(),
    }
    for mm in range(N_CORES):
        im = dict(common)
        im["xT_q"] = np.ascontiguousarray(xpad[node_at[mm]].T).astype(ml_dtypes.bfloat16)
        im["src_tab"] = src_tabs[mm]
        im["deg_tab"] = deg_tabs[mm]
        in_maps.append(im)

    cfg = dict(n=n, n_pad=n_pad, n_c=n_c, d_sched=d_sched, c_tot=c_tot,
               node_at=node_at)
    return in_maps, cfg


def get_program(cfg, finalize=True):
    key = (cfg["n_pad"], cfg["n_c"], cfg["d_sched"])
    if key not in _PROGRAM_CACHE:
        nc = _build_program(cfg["n_pad"], cfg["n_c"], cfg["d_sched"],
                            cfg["c_tot"])
        if finalize:
            nc.finalize()
        _PROGRAM_CACHE[key] = nc
    return _PROGRAM_CACHE[key]


def assemble(results, cfg):
    n, n_c = cfg["n"], cfg["n_c"]
    out_full = np.empty((n, DIM), dtype=np.float32)
    for mm in range(N_CORES):
        nodes = cfg["node_at"][mm]
        valid = nodes < n
        out_full[nodes[valid]] = np.asarray(results[mm]["out"])[valid]
    return out_full


LAST_RESULT = None


def kernel(**inputs):
    global LAST_RESULT
    from concourse.bass_utils import run_bass_kernel_spmd

    in_maps, cfg = prepare(**inputs)
    nc = get_program(cfg)
    res = run_bass_kernel_spmd(nc, in_maps, core_ids=list(range(N_CORES)))
    LAST_RESULT = res
    return assemble(res.results, cfg)


# revision 8
# speedup vs baseline: 1.1583x; 1.1583x over previous
"""GAT layer (4 heads, 128 dim) on 8 Trainium2 NeuronCores.

Strategy (edge-parallel over dst, degree-sorted, batched HW gather):
  - Host relabels nodes by descending (lo-degree, hi-degree) and deals them
    round-robin to the 8 cores, so every core sees an identical degree
    profile and the SPMD program (one Bass module, per-core data) bakes a
    shared per-window slot schedule with small padding.
  - K|V rows (bf16, 512B) live in two DRAM tables (node id < 32768 and the
    rest) so row indices fit the int16 index format of the batched
    dma_gather (InstDMAGatherAnt) instruction; each 128-node window issues
    one gather per table half for all its edge slots.
  - Per window: per-edge scores q.k/sqrt(d) via broadcast-multiply +
    strided reduce, exp (global-max shift of the reference cancels in the
    normalization up to ~1e-8), padding masked by per-node degree, message
    aggregation and score sums via strided reductions, then
    out = relu(((sum exp*V)/(sum exp + 1e-8)) @ Wo^T + bo).
  - No collectives: each core owns a disjoint slice of output rows; the
    host scatters per-core outputs back through the permutation.
"""

import os
import sys

for _p in ("/opt/trn_rl_repo", "/opt/pypackages"):
    if _p not in sys.path:
        sys.path.append(_p)

import numpy as np
import ml_dtypes

P = 128
N_CORES = 8
DIM = 128
NUM_HEADS = 4
HEAD_DIM = 32
INV_SQRT_HD = 1.0 / np.sqrt(HEAD_DIM).astype(np.float32)
EPS = 1e-8
D_CH = 16          # edge slots per compute chunk
PH1_CHUNK = 512    # nodes per phase-1 x-chunk
LO = 32768         # node-id split for int16 gather indices

_PROGRAM_CACHE = {}


def _chunks(d):
    out = []
    s = 0
    while s < d:
        out.append((s, min(D_CH, d - s)))
        s += D_CH
    return out


def _build_program(n_pad, n_c, d_sched, c_idx):
    import concourse.bass as bass
    import concourse.bacc as bacc
    import concourse.mybir as mybir
    from concourse.tile import TileContext
    from concourse.masks import make_identity

    f32 = mybir.dt.float32
    bf16 = mybir.dt.bfloat16
    i16 = mybir.dt.int16
    n_w = len(d_sched)
    d_max = max(max(dl, dh) for dl, dh in d_sched)
    d_max = max(d_max, D_CH)

    nc = bacc.Bacc()
    xT_full = nc.dram_tensor("xT_full", [P, n_pad], bf16, kind="ExternalInput")
    xT_q = nc.dram_tensor("xT_q", [P, n_c], bf16, kind="ExternalInput")
    w_qT = nc.dram_tensor("w_qT", [P, DIM], bf16, kind="ExternalInput")
    w_kT = nc.dram_tensor("w_kT", [P, DIM], bf16, kind="ExternalInput")
    w_vT = nc.dram_tensor("w_vT", [P, DIM], bf16, kind="ExternalInput")
    w_oT = nc.dram_tensor("w_oT", [P, DIM], bf16, kind="ExternalInput")
    b_q = nc.dram_tensor("b_q", [P, DIM], f32, kind="ExternalInput")
    b_k = nc.dram_tensor("b_k", [P, DIM], f32, kind="ExternalInput")
    b_v = nc.dram_tensor("b_v", [P, DIM], f32, kind="ExternalInput")
    b_o = nc.dram_tensor("b_o", [P, DIM], f32, kind="ExternalInput")
    idx_tab = nc.dram_tensor("idx_tab", [P, max(c_idx, 8)], i16,
                             kind="ExternalInput")
    deg_lo = nc.dram_tensor("deg_lo", [P, n_w], f32, kind="ExternalInput")
    deg_hi = nc.dram_tensor("deg_hi", [P, n_w], f32, kind="ExternalInput")
    out = nc.dram_tensor("out", [n_c, DIM], f32, kind="ExternalOutput")
    kv_lo = nc.dram_tensor("kv_lo", [min(LO, n_pad), 2 * DIM], bf16)
    kv_hi = nc.dram_tensor("kv_hi", [max(n_pad - LO, P), 2 * DIM], bf16)

    with TileContext(nc) as tc:
        with (
            tc.tile_pool(name="consts", bufs=1) as cp,
            tc.tile_pool(name="ph1", bufs=3) as p1,
            tc.tile_pool(name="ph1ps", bufs=2, space="PSUM") as p1ps,
            tc.tile_pool(name="kvgp", bufs=2) as kvp,
            tc.tile_pool(name="win", bufs=3) as wp,
            tc.tile_pool(name="winacc", bufs=2) as ap,
            tc.tile_pool(name="winps", bufs=1, space="PSUM") as pp,
        ):
            # ---- constants ----
            wq_sb = cp.tile([P, DIM], bf16, tag="wq")
            wk_sb = cp.tile([P, DIM], bf16, tag="wk")
            wv_sb = cp.tile([P, DIM], bf16, tag="wv")
            wo_sb = cp.tile([P, DIM], bf16, tag="wo")
            bq_sb = cp.tile([P, DIM], f32, tag="bq")
            bk_sb = cp.tile([P, DIM], f32, tag="bk")
            bv_sb = cp.tile([P, DIM], f32, tag="bv")
            bo_sb = cp.tile([P, DIM], f32, tag="bo")
            nc.sync.dma_start(out=wq_sb[:], in_=w_qT[:])
            nc.sync.dma_start(out=wk_sb[:], in_=w_kT[:])
            nc.sync.dma_start(out=wv_sb[:], in_=w_vT[:])
            nc.sync.dma_start(out=wo_sb[:], in_=w_oT[:])
            nc.sync.dma_start(out=bq_sb[:], in_=b_q[:])
            nc.sync.dma_start(out=bk_sb[:], in_=b_k[:])
            nc.sync.dma_start(out=bv_sb[:], in_=b_v[:])
            nc.sync.dma_start(out=bo_sb[:], in_=b_o[:])
            idx_sb = cp.tile([P, max(c_idx, 8)], i16, tag="idx")
            dlo_sb = cp.tile([P, n_w], f32, tag="dlo")
            dhi_sb = cp.tile([P, n_w], f32, tag="dhi")
            nc.sync.dma_start(out=idx_sb[:], in_=idx_tab[:])
            nc.sync.dma_start(out=dlo_sb[:], in_=deg_lo[:])
            nc.sync.dma_start(out=dhi_sb[:], in_=deg_hi[:])
            xq_sb = cp.tile([P, n_c], bf16, tag="xq")
            nc.sync.dma_start(out=xq_sb[:], in_=xT_q[:])
            ident = cp.tile([P, P], f32, tag="ident")
            make_identity(nc, ident[:])
            iota_i = cp.tile([P, d_max], mybir.dt.int32, tag="iotai")
            nc.gpsimd.iota(iota_i[:], pattern=[[1, d_max]], base=0,
                           channel_multiplier=0)
            iota_f = cp.tile([P, d_max], f32, tag="iotaf")
            nc.vector.tensor_copy(out=iota_f[:], in_=iota_i[:])
            relu_bo = cp.tile([P, DIM], f32, tag="relubo")
            nc.scalar.activation(out=relu_bo[:], in_=bo_sb[:],
                                 func=mybir.ActivationFunctionType.Relu)

            # ---- phase 1: K|V tables ----
            for c0 in range(0, n_pad, PH1_CHUNK):
                cw = min(PH1_CHUNK, n_pad - c0)
                xc = p1.tile([P, cw], bf16, tag="xc")
                nc.sync.dma_start(out=xc[:], in_=xT_full[:, c0:c0 + cw])
                for s0 in range(0, cw, P):
                    kv_sb = p1.tile([P, 2 * DIM], bf16, tag="kvsb")
                    ps_k = p1ps.tile([P, DIM], f32, tag="psk")
                    ps_v = p1ps.tile([P, DIM], f32, tag="psv")
                    lhs = xc[:, s0:s0 + P]
                    nc.tensor.matmul(out=ps_k[:], lhsT=lhs, rhs=wk_sb[:],
                                     start=True, stop=True)
                    nc.tensor.matmul(out=ps_v[:], lhsT=lhs, rhs=wv_sb[:],
                                     start=True, stop=True)
                    nc.vector.tensor_tensor(out=kv_sb[:, 0:DIM], in0=ps_k[:],
                                            in1=bk_sb[:], op=mybir.AluOpType.add)
                    nc.scalar.copy(out=kv_sb[:, DIM:2 * DIM], in_=ps_v[:])
                    node0 = c0 + s0
                    if node0 < LO:
                        nc.sync.dma_start(out=kv_lo[node0:node0 + P, :],
                                          in_=kv_sb[:])
                    else:
                        nc.sync.dma_start(out=kv_hi[node0 - LO:node0 - LO + P, :],
                                          in_=kv_sb[:])

            tc.strict_bb_all_engine_barrier()

            # ---- phase 2: windows ----
            icol = 0
            for w in range(n_w):
                d_lo, d_hi = d_sched[w]
                d_tot = d_lo + d_hi
                row0 = w * P
                if d_tot == 0:
                    nc.sync.dma_start(out=out[row0:row0 + P, :], in_=relu_bo[:])
                    continue

                # q_w = xq[:, window] @ WqT + bq   (node-major, bf16)
                ps_q = pp.tile([P, DIM], f32, tag="psq")
                nc.tensor.matmul(out=ps_q[:], lhsT=xq_sb[:, row0:row0 + P],
                                 rhs=wq_sb[:], start=True, stop=True)
                q_w = wp.tile([P, DIM], bf16, tag="qw")
                nc.vector.tensor_tensor(out=q_w[:], in0=ps_q[:], in1=bq_sb[:],
                                        op=mybir.AluOpType.add)

                agg = ap.tile([P, DIM], f32, tag="agg")
                ssum = ap.tile([P, NUM_HEADS], f32, tag="ssum")
                nc.vector.memset(agg[:], 0.0)
                nc.vector.memset(ssum[:], 0.0)

                kv_g = kvp.tile([P, d_max * 2 * 2 * DIM], bf16, tag="kvg")
                segs = []
                if d_lo:
                    segs.append((0, d_lo, dlo_sb))
                if d_hi:
                    segs.append((d_lo, d_hi, dhi_sb))
                for (sbase, dseg, dtab) in segs:
                    tabl = kv_lo if dtab is dlo_sb else kv_hi
                    ni = dseg * P
                    nc.gpsimd.dma_gather(
                        out_ap=kv_g[:, sbase * 2 * DIM:(sbase + dseg) * 2 * DIM]
                            .rearrange("p (c e) -> p c e", e=2 * DIM),
                        in_ap=tabl[:],
                        idxs_ap=idx_sb[:, icol:icol + ni // 16],
                        num_idxs=ni,
                        num_idxs_reg=ni,
                        elem_size=2 * DIM,
                        single_packet=False,
                    )
                    icol += ni // 16

                for (sbase, dseg, dtab) in segs:
                    for (c0, cl) in _chunks(dseg):
                        s0 = sbase + c0
                        kv3 = kv_g[:, s0 * 2 * DIM:(s0 + cl) * 2 * DIM] \
                            .rearrange("p (s c) -> p s c", c=2 * DIM)

                        tmul = wp.tile([P, D_CH * DIM], bf16, tag="tmul")
                        t3 = tmul[:, :cl * DIM].rearrange(
                            "p (s f) -> p s f", f=DIM)
                        nc.vector.tensor_tensor(
                            out=t3, in0=kv3[:, :, 0:DIM],
                            in1=q_w[:, None, :].broadcast_to([P, cl, DIM]),
                            op=mybir.AluOpType.mult)
                        scr = wp.tile([P, D_CH * NUM_HEADS], f32, tag="scr")
                        nc.vector.tensor_reduce(
                            out=scr[:, :cl * NUM_HEADS],
                            in_=tmul[:, :cl * DIM].rearrange(
                                "p (s h d) -> p s h d",
                                h=NUM_HEADS, d=HEAD_DIM),
                            op=mybir.AluOpType.add, axis=mybir.AxisListType.X)

                        exps = wp.tile([P, D_CH * NUM_HEADS], f32, tag="exps")
                        nc.scalar.activation(
                            out=exps[:, :cl * NUM_HEADS],
                            in_=scr[:, :cl * NUM_HEADS],
                            func=mybir.ActivationFunctionType.Exp,
                            scale=float(INV_SQRT_HD))
                        mask = wp.tile([P, D_CH], f32, tag="mask")
                        nc.gpsimd.tensor_scalar(
                            out=mask[:, :cl], in0=iota_f[:, c0:c0 + cl],
                            scalar1=dtab[:, w:w + 1], scalar2=None,
                            op0=mybir.AluOpType.is_lt)
                        expm = wp.tile([P, D_CH * NUM_HEADS], f32, tag="expm")
                        nc.vector.tensor_tensor(
                            out=expm[:, :cl * NUM_HEADS].rearrange(
                                "p (s h) -> p s h", h=NUM_HEADS),
                            in0=exps[:, :cl * NUM_HEADS].rearrange(
                                "p (s h) -> p s h", h=NUM_HEADS),
                            in1=mask[:, :cl, None].broadcast_to(
                                [P, cl, NUM_HEADS]),
                            op=mybir.AluOpType.mult)

                        ssc = wp.tile([P, NUM_HEADS], f32, tag="ssc")
                        nc.vector.tensor_reduce(
                            out=ssc[:],
                            in_=expm[:, :cl * NUM_HEADS].rearrange(
                                "p (s h) -> p s h",
                                h=NUM_HEADS).transpose([0, 2, 1]),
                            op=mybir.AluOpType.add, axis=mybir.AxisListType.X)
                        nc.vector.tensor_tensor(out=ssum[:], in0=ssum[:],
                                                in1=ssc[:],
                                                op=mybir.AluOpType.add)

                        msm = wp.tile([P, D_CH * DIM], bf16, tag="msm")
                        m4 = msm[:, :cl * DIM].rearrange(
                            "p (s h d) -> p s h d", h=NUM_HEADS, d=HEAD_DIM)
                        nc.vector.tensor_tensor(
                            out=m4,
                            in0=kv3[:, :, DIM:2 * DIM].rearrange(
                                "p s (h d) -> p s h d", d=HEAD_DIM),
                            in1=expm[:, :cl * NUM_HEADS].rearrange(
                                "p (s h) -> p s h", h=NUM_HEADS)[:, :, :, None]
                                .broadcast_to([P, cl, NUM_HEADS, HEAD_DIM]),
                            op=mybir.AluOpType.mult)
                        agc = wp.tile([P, DIM], f32, tag="agc")
                        nc.vector.tensor_reduce(
                            out=agc[:],
                            in_=msm[:, :cl * DIM].rearrange(
                                "p (s f) -> p s f", f=DIM).transpose([0, 2, 1]),
                            op=mybir.AluOpType.add, axis=mybir.AxisListType.X)
                        nc.vector.tensor_tensor(out=agg[:], in0=agg[:],
                                                in1=agc[:],
                                                op=mybir.AluOpType.add)

                # V-bias correction: agg += ssum (x) bv   (bv folded out of
                # phase 1; bv_rep rows are identical so per-head broadcast ok)
                bvc = wp.tile([P, DIM], f32, tag="bvc")
                nc.vector.tensor_tensor(
                    out=bvc[:].rearrange("p (h d) -> p h d", d=HEAD_DIM),
                    in0=bv_sb[:].rearrange("p (h d) -> p h d", d=HEAD_DIM),
                    in1=ssum[:, :, None].broadcast_to([P, NUM_HEADS, HEAD_DIM]),
                    op=mybir.AluOpType.mult)
                nc.vector.tensor_tensor(out=agg[:], in0=agg[:], in1=bvc[:],
                                        op=mybir.AluOpType.add)

                # normalize: agg / (ssum + eps), per head
                inv4 = wp.tile([P, NUM_HEADS], f32, tag="inv4")
                nc.vector.tensor_scalar(
                    out=inv4[:], in0=ssum[:], scalar1=float(EPS), scalar2=None,
                    op0=mybir.AluOpType.add)
                nc.vector.reciprocal(out=inv4[:], in_=inv4[:])
                aggn = wp.tile([P, DIM], f32, tag="aggn")
                nc.vector.tensor_tensor(
                    out=aggn[:].rearrange("p (h d) -> p h d", d=HEAD_DIM),
                    in0=agg[:].rearrange("p (h d) -> p h d", d=HEAD_DIM),
                    in1=inv4[:, :, None].broadcast_to([P, NUM_HEADS, HEAD_DIM]),
                    op=mybir.AluOpType.mult)

                # out = relu(aggn @ WoT + bo)
                ps_t = pp.tile([P, DIM], f32, tag="pst")
                nc.tensor.transpose(out=ps_t[:], in_=aggn[:], identity=ident[:])
                aggT = wp.tile([P, DIM], bf16, tag="aggT")
                nc.scalar.copy(out=aggT[:], in_=ps_t[:])
                ps_o = pp.tile([P, DIM], f32, tag="pso")
                nc.tensor.matmul(out=ps_o[:], lhsT=aggT[:], rhs=wo_sb[:],
                                 start=True, stop=True)
                res = wp.tile([P, DIM], f32, tag="res")
                nc.vector.tensor_tensor(out=res[:], in0=ps_o[:], in1=bo_sb[:],
                                        op=mybir.AluOpType.add)
                res2 = wp.tile([P, DIM], f32, tag="res2")
                nc.scalar.activation(out=res2[:], in_=res[:],
                                     func=mybir.ActivationFunctionType.Relu)
                nc.sync.dma_start(out=out[row0:row0 + P, :], in_=res2[:])

    return nc


def prepare(x, edge_index, Wq, bq, Wk, bk, Wv, bv, Wo, bo):
    """Host-side layout prep: permutation, dealing, slot tables. No math."""
    n = x.shape[0]
    e = edge_index.shape[1]
    n_c = -(-n // (N_CORES * P)) * P
    n_pad = N_CORES * n_c
    n_w = n_c // P

    src = np.asarray(edge_index[0], dtype=np.int64)
    dst = np.asarray(edge_index[1], dtype=np.int64)
    is_hi = src >= LO
    dlo = np.bincount(dst[~is_hi], minlength=n_pad).astype(np.int64)
    dhi = np.bincount(dst[is_hi], minlength=n_pad).astype(np.int64)

    order = np.lexsort((-dhi, -dlo))                 # rank -> node
    rank_of = np.empty(n_pad, dtype=np.int64)
    rank_of[order] = np.arange(n_pad)

    node_at = order.reshape(n_c, N_CORES).T          # [core, pos] -> node
    d_sched = []
    for w in range(n_w):
        sl = order[w * P * N_CORES:(w + 1) * P * N_CORES]
        d_sched.append((int(dlo[sl].max()), int(dhi[sl].max())))
    d_sched = tuple(d_sched)

    # per-core int16 gather index tables, [16, .] wrap replicated to 128 rows
    c_idx = sum((dl + dh) * P // 16 for dl, dh in d_sched)
    idx_tabs = np.zeros((N_CORES, 128, max(c_idx, 8)), dtype=np.int16)

    # edges grouped by (dst, half): stable sort by dst then half
    half_key = is_hi.astype(np.int64)
    eo = np.lexsort((half_key, dst))
    dst_s, src_s, hi_s = dst[eo], src[eo], half_key[eo]
    # intra-(node,half) slot index
    starts = np.zeros(n_pad + 1, dtype=np.int64)
    np.cumsum(dlo + dhi, out=starts[1:])
    pos_in_node = np.arange(e) - starts[dst_s]
    j_lo = pos_in_node                                # slot within lo block
    j_hi = pos_in_node - dlo[dst_s]                   # slot within hi block
    slot = np.where(hi_s == 1, j_hi, j_lo)

    r = rank_of[dst_s]
    m = r % N_CORES
    posn = r // N_CORES
    w_arr = posn // P
    p_arr = posn % P

    # column offset of each (window, half) idx block, in int16 columns
    blk_off = np.zeros((n_w, 2), dtype=np.int64)
    acc = 0
    for w, (dl, dh) in enumerate(d_sched):
        blk_off[w, 0] = acc
        acc += dl * P // 16
        blk_off[w, 1] = acc
        acc += dh * P // 16
    # j within gather = slot*128 + p  ->  idx col = off + j//16, row j%16
    j_g = slot * P + p_arr
    col = blk_off[w_arr, hi_s] + j_g // 16
    row = j_g % 16
    val = np.where(hi_s == 1, src_s - LO, src_s).astype(np.int16)
    flat = idx_tabs.reshape(N_CORES, -1)
    width = idx_tabs.shape[2]
    flat[m, row * width + col] = val
    idx_tabs[:, 16:, :] = np.tile(idx_tabs[:, :16, :], (1, 7, 1))

    deg_lo_t = np.ascontiguousarray(
        dlo[node_at].reshape(N_CORES, n_w, P).transpose(0, 2, 1)
    ).astype(np.float32)
    deg_hi_t = np.ascontiguousarray(
        dhi[node_at].reshape(N_CORES, n_w, P).transpose(0, 2, 1)
    ).astype(np.float32)

    xpad = np.zeros((n_pad, DIM), dtype=np.float32)
    xpad[:n] = np.asarray(x, dtype=np.float32)
    xT_full = np.ascontiguousarray(xpad.T).astype(ml_dtypes.bfloat16)

    in_maps = []
    common = {
        "xT_full": xT_full,
        "w_qT": np.ascontiguousarray(np.asarray(Wq, np.float32).T).astype(ml_dtypes.bfloat16),
        "w_kT": np.ascontiguousarray(np.asarray(Wk, np.float32).T).astype(ml_dtypes.bfloat16),
        "w_vT": np.ascontiguousarray(np.asarray(Wv, np.float32).T).astype(ml_dtypes.bfloat16),
        "w_oT": np.ascontiguousarray(np.asarray(Wo, np.float32).T).astype(ml_dtypes.bfloat16),
        "b_q": np.broadcast_to(np.asarray(bq, np.float32), (P, DIM)).copy(),
        "b_k": np.broadcast_to(np.asarray(bk, np.float32), (P, DIM)).copy(),
        "b_v": np.broadcast_to(np.asarray(bv, np.float32), (P, DIM)).copy(),
        "b_o": np.broadcast_to(np.asarray(bo, np.float32), (P, DIM)).copy(),
    }
    for mm in range(N_CORES):
        im = dict(common)
        im["xT_q"] = np.ascontiguousarray(xpad[node_at[mm]].T).astype(ml_dtypes.bfloat16)
        im["idx_tab"] = idx_tabs[mm]
        im["deg_lo"] = deg_lo_t[mm]
        im["deg_hi"] = deg_hi_t[mm]
        in_maps.append(im)

    cfg = dict(n=n, n_pad=n_pad, n_c=n_c, d_sched=d_sched, c_idx=c_idx,
               node_at=node_at)
    return in_maps, cfg


def get_program(cfg, finalize=True):
    key = (cfg["n_pad"], cfg["n_c"], cfg["d_sched"])
    if key not in _PROGRAM_CACHE:
        nc = _build_program(cfg["n_pad"], cfg["n_c"], cfg["d_sched"],
                            cfg["c_idx"])
        if finalize:
            nc.finalize()
        _PROGRAM_CACHE[key] = nc
    return _PROGRAM_CACHE[key]


def assemble(results, cfg):
    n = cfg["n"]
    out_full = np.empty((n, DIM), dtype=np.float32)
    for mm in range(N_CORES):
        nodes = cfg["node_at"][mm]
        valid = nodes < n
        out_full[nodes[valid]] = np.asarray(results[mm]["out"])[valid]
    return out_full


LAST_RESULT = None


def kernel(**inputs):
    global LAST_RESULT
    from concourse.bass_utils import run_bass_kernel_spmd

    in_maps, cfg = prepare(**inputs)
    nc = get_program(cfg)
    res = run_bass_kernel_spmd(nc, in_maps, core_ids=list(range(N_CORES)))
    LAST_RESULT = res
    return assemble(res.results, cfg)
